# revision 1
# baseline (speedup 1.0000x reference)
"""BertSelfAttention (relative_key_query) Trainium2 Bass kernel, 8-core SPMD.

Sharding: 32 (batch, head) pairs -> core c handles batch c//4, heads
[4*(c%4), 4*(c%4)+4).  Each core runs an identical program on its own
input shard; host assembles the full [2, 2048, 1024] output.

Math (per core, per head), in "mirrored" coordinates a = 2047 - r, b = l:
  scores[a, b] = k'[a].q[b] + (k'[a] + q[b]) . E[a + b]           (E = dist_emb)
  probs        = exp(scores/8) * em'[a]  (em' = exp(mask) reversed), normalized
  ctx[b, d]    = sum_a probs[a, b] v'[a, d]                       (v' = v reversed)

The relative-position terms use the band-matmul + HBM "shear" trick:
  band[i, m] = src[A0+i] . E[A0+m]   (a plain matmul, [128 x 2175] per tile)
written to scratch DRAM with row stride 2176; the shifted-diagonal view
  rel[i, x] = band[i, i+x]  (per-partition shift)
is then a *plain strided read*: flat offset i*2177 + x.
"""

import sys

sys.path.insert(0, "/opt/trn_rl_repo")

import numpy as np

B, S, HID = 2, 2048, 1024
H, D = 16, 64
MAXPOS = 2048
NCORES = 8
HPC = 4  # heads per core

KT = 8  # 1024 / 128 contraction chunks for projections
MT = 2  # 256 / 128 output chunks for projections
NB = 4  # 2048 / 512 b-chunks
AT = 16  # 2048 / 128 a-tiles
BW = 2176  # band width (padded even; valid data in [0, 2175))
BR = 2176  # band row stride in scratch (shear: read stride BR+1)
BCHS = [512, 512, 512, 512, 128]  # band N-chunks (even sizes for fp32r)

_CACHE = {}


def _build():
    if "nc" in _CACHE:
        return _CACHE["nc"]

    import concourse.bass as bass
    import concourse.tile as tile
    from concourse import bacc, mybir
    from concourse.masks import make_identity

    dt = mybir.dt
    f32, bf16, fp8 = dt.float32, dt.bfloat16, dt.float8e4
    fp16 = dt.float16
    AF = mybir.ActivationFunctionType

    nc = bacc.Bacc(
        "TRN2", target_bir_lowering=False, debug=False, enable_asserts=True
    )

    hT = nc.declare_dram_parameter("hT", [HID, S], fp16, isOutput=False)
    hTr = nc.declare_dram_parameter("hTr", [HID, S], fp16, isOutput=False)
    # host pre-swizzled: [128, KT*256], chunk kc at cols [256*kc, 256*(kc+1))
    wqT = nc.declare_dram_parameter("wqT", [128, KT * 256], fp16, isOutput=False)
    wkT = nc.declare_dram_parameter("wkT", [128, KT * 256], fp16, isOutput=False)
    wvT = nc.declare_dram_parameter("wvT", [128, KT * 256], fp16, isOutput=False)
    bq = nc.declare_dram_parameter("bq", [128, MT], f32, isOutput=False)
    bk = nc.declare_dram_parameter("bk", [128, MT], f32, isOutput=False)
    bv = nc.declare_dram_parameter("bv", [128, MT], f32, isOutput=False)
    ET = nc.declare_dram_parameter("ET", [D, 2 * MAXPOS], fp16, isOutput=False)
    emr = nc.declare_dram_parameter("emr", [128, AT], f32, isOutput=False)
    ctxT = nc.declare_dram_parameter("ctxT", [256, S], f32, isOutput=True)

    # scratch: per head, per term: 16 tiles x [128 x BW] at row stride BR
    scr = [
        [nc.dram_tensor(f"scr_{h}_{t}", [AT * 128 * BR], fp8) for t in range(2)]
        for h in range(HPC)
    ]

    with tile.TileContext(nc) as tc:
        from contextlib import ExitStack

        with ExitStack() as ctx:
            persist = ctx.enter_context(tc.tile_pool(name="persist", bufs=1))

            # ---- constants ----
            # E^T duplicated into both partition halves so 64-row operands
            # based at partition 0 or 64 both find it at their own base
            et_sb = persist.tile([128, 2 * MAXPOS], fp16, tag="et")
            nc.sync.dma_start(et_sb[0:D, :], ET[:])
            nc.sync.dma_start(et_sb[D : 2 * D, :], ET[:])
            emr_sb = persist.tile([128, AT], f32, tag="emr")
            nc.sync.dma_start(emr_sb[:], emr[:])
            i128 = persist.tile([128, 128], fp16, tag="i128")
            make_identity(nc, i128[:])
            i128f = persist.tile([128, 128], fp16, tag="i128f")
            make_identity(nc, i128f[:])
            bias_sb = {}
            for nm, t in (("q", bq), ("k", bk), ("v", bv)):
                bias_sb[nm] = persist.tile([128, MT], f32, tag=f"b{nm}", name=f"bias_{nm}")
                nc.sync.dma_start(bias_sb[nm][:], t[:])

            # ---- projections: qT = wqT.T @ hT (+bias), etc. ----
            # [256, S] stored as [128, MT*S]: M-chunk m at cols [m*S, (m+1)*S)
            qT = persist.tile([128, MT * S], fp16, tag="qT")
            kT = persist.tile([128, MT * S], fp16, tag="kT")
            vT = persist.tile([128, MT * S], fp16, tag="vT")

            with tc.tile_pool(name="proj_ps", bufs=1, space="PSUM") as proj_ps, \
                 tc.tile_pool(name="hload", bufs=3) as hload:
                for nm, wt, src, dst in (
                    ("q", wqT, hT, qT),
                    ("k", wkT, hTr, kT),
                    ("v", wvT, hTr, vT),
                ):
                    # load full weight [1024, 256] as [128, 8*256] partition-major
                    w_sb = hload.tile([128, KT * 256], fp16, tag="w")
                    nc.sync.dma_start(w_sb[:], wt[:])
                    ps = [
                        [
                            proj_ps.tile(
                                [128, 512], mybir.dt.float32,
                                name=f"ps_{nm}_{m}_{n}", tag=f"ps_{m}_{n}",
                            )
                            for n in range(NB)
                        ]
                        for m in range(MT)
                    ]
                    for kc in range(KT):
                        h_sb = hload.tile([128, S], fp16, tag="h")
                        nc.sync.dma_start(h_sb[:], src[128 * kc : 128 * (kc + 1), :])
                        for m in range(MT):
                            for n in range(NB):
                                nc.tensor.matmul(
                                    ps[m][n][:],
                                    w_sb[:, kc * 256 + 128 * m : kc * 256 + 128 * (m + 1)],
                                    h_sb[:, 512 * n : 512 * (n + 1)],
                                    start=(kc == 0),
                                    stop=(kc == KT - 1),
                                )
                    for m in range(MT):
                        for n in range(NB):
                            # bias add (kept general; zero in practice)
                            nc.vector.tensor_scalar_add(
                                ps[m][n][:], ps[m][n][:], bias_sb[nm][:, m : m + 1]
                            )
                            d_ap = dst[:, m * S + 512 * n : m * S + 512 * (n + 1)]
                            if n % 2 == 0:
                                nc.vector.tensor_copy(d_ap, ps[m][n][:])
                            else:
                                nc.scalar.copy(d_ap, ps[m][n][:])

            # ---- per-head attention ----
            with tc.tile_pool(name="bpsum", bufs=2, space="PSUM") as bpsum, \
                 tc.tile_pool(name="spsum", bufs=2, space="PSUM") as spsum, \
                 tc.tile_pool(name="ctxps", bufs=1, space="PSUM") as ctxps, \
                 tc.tile_pool(name="bstage", bufs=3) as bstage, \
                 tc.tile_pool(name="termA", bufs=2) as termA_pool, \
                 tc.tile_pool(name="termB", bufs=1) as termB_pool, \
                 tc.tile_pool(name="probs", bufs=2) as probs_pool, \
                 tc.tile_pool(name="vaug", bufs=2) as vaug_pool, \
                 tc.tile_pool(name="epi", bufs=4) as epi:
                for hl in range(HPC):
                    p0 = 64 * (hl % 2)
                    c0 = (hl // 2) * S
                    qh = qT[p0 : p0 + 64, c0 : c0 + S]
                    kh = kT[p0 : p0 + 64, c0 : c0 + S]
                    vh = vT[p0 : p0 + 64, c0 : c0 + S]

                    # v_aug[a, 0:64] = v'[a, :] * em'[a];  v_aug[a, 64] = em'[a]
                    vaug = vaug_pool.tile([128, AT * 65], fp16, tag="vaug")
                    for at in range(AT):
                        vps = bpsum.tile([128, 512], fp16, tag="bp", name=f"vps_{hl}_{at}")
                        nc.tensor.transpose(
                            vps[:, 0:D],
                            vh[:, 128 * at : 128 * (at + 1)],
                            i128f[p0 : p0 + D, p0 : p0 + D],
                        )
                        nc.vector.tensor_scalar_mul(
                            vaug[:, 65 * at : 65 * at + 64],
                            vps[:, 0:D],
                            emr_sb[:, at : at + 1],
                        )
                        nc.vector.tensor_copy(
                            vaug[:, 65 * at + 64 : 65 * at + 65],
                            emr_sb[:, at : at + 1],
                        )

                    # bands -> scratch (term 0: k'-band, term 1: q-band)
                    for term, srcT in ((0, kh), (1, qh)):
                        for t in range(AT):
                            stage = bstage.tile([128, BW], fp16, tag="bs")
                            a0 = 128 * t
                            c0 = 0
                            for chi, bch in enumerate(BCHS):
                                bps = bpsum.tile([128, 512], mybir.dt.float32, tag="bp")
                                nc.tensor.matmul(
                                    bps[:, 0:bch],
                                    srcT[:, a0 : a0 + 128],
                                    et_sb[p0 : p0 + D, a0 + c0 : a0 + c0 + bch],
                                    start=True,
                                    stop=True,
                                )
                                if chi % 2 == 0:
                                    nc.vector.tensor_copy(
                                        stage[:, c0 : c0 + bch], bps[:, 0:bch]
                                    )
                                else:
                                    nc.scalar.copy(
                                        stage[:, c0 : c0 + bch], bps[:, 0:bch]
                                    )
                                c0 += bch
                            nc.gpsimd.dma_start(
                                bass.AP(scr[hl][term], 128 * BR * t, [[BR, 128], [1, BW]]),
                                stage[:],
                            )

                    # term-B tiles resident for the whole a-loop
                    termB = termB_pool.tile([128, AT * 2048], fp8, tag="termB")
                    for j in range(AT):
                        nc.sync.dma_start(
                            termB[:, 2048 * j : 2048 * (j + 1)],
                            bass.AP(scr[hl][1], 128 * BR * j, [[BR + 1, 128], [1, 2048]]),
                        )

                    ctx_ps = [
                        ctxps.tile(
                            [65, 512], mybir.dt.float32,
                            name=f"ctx_{hl}_{n}", tag=f"ctx_{n}",
                        )
                        for n in range(NB)
                    ]
                    for at in range(AT):
                        a0 = 128 * at
                        tA = termA_pool.tile([128, 2048], fp8, tag="tA")
                        nc.sync.dma_start(
                            tA[:],
                            bass.AP(scr[hl][0], 128 * BR * at, [[BR + 1, 128], [1, 2048]]),
                        )
                        probs = probs_pool.tile([128, 2048], fp16, tag="pr")
                        for n in range(NB):
                            sp = spsum.tile([128, 512], mybir.dt.float32, tag="s")
                            # transposed rel-q blocks (start=True zeroes each slice)
                            for jj in range(4):
                                j = 4 * n + jj
                                nc.tensor.matmul(
                                    sp[:, 128 * jj : 128 * (jj + 1)],
                                    termB[:, 2048 * j + a0 : 2048 * j + a0 + 128],
                                    i128[:],
                                    start=True,
                                    stop=False,
                                    skip_group_check=True,
                                )
                            # + rel-k (aligned)
                            nc.tensor.matmul(
                                sp[:],
                                i128[:],
                                tA[:, 512 * n : 512 * (n + 1)],
                                start=False,
                                stop=False,
                                skip_group_check=True,
                            )
                            # + k'.q
                            nc.tensor.matmul(
                                sp[:],
                                kh[:, a0 : a0 + 128],
                                qh[:, 512 * n : 512 * (n + 1)],
                                start=False,
                                stop=True,
                                skip_group_check=True,
                            )
                            nc.scalar.activation(
                                probs[:, 512 * n : 512 * (n + 1)],
                                sp[:],
                                AF.Exp,
                                scale=0.125,
                            )
                            nc.tensor.matmul(
                                ctx_ps[n][:],
                                vaug[:, 65 * at : 65 * (at + 1)],
                                probs[:, 512 * n : 512 * (n + 1)],
                                start=(at == 0),
                                stop=(at == AT - 1),
                            )

                    # normalize + write out
                    for n in range(NB):
                        den = epi.tile([1, 512], f32, tag="den")
                        nc.vector.tensor_copy(den[:], ctx_ps[n][64:65, :])
                        rec = epi.tile([1, 512], f32, tag="rec")
                        nc.vector.reciprocal(rec[:], den[:])
                        rec64 = epi.tile([64, 512], f32, tag="rec64")
                        nc.gpsimd.partition_broadcast(rec64[:], rec[:])
                        o = epi.tile([64, 512], f32, tag="o")
                        nc.vector.tensor_tensor(
                            o[:],
                            ctx_ps[n][0:64, :],
                            rec64[:],
                            mybir.AluOpType.mult,
                        )
                        nc.sync.dma_start(
                            ctxT[64 * hl : 64 * (hl + 1), 512 * n : 512 * (n + 1)],
                            o[:],
                        )

    nc.compile()
    _CACHE["nc"] = nc
    return nc


def _swz(w):
    # W block [256, 1024] -> W.T [1024, 256] -> [8, 128, 256] -> [128, 8*256]
    wT = np.ascontiguousarray(w.T).reshape(KT, 128, 256)
    return np.ascontiguousarray(wT.transpose(1, 0, 2).reshape(128, KT * 256)).astype(np.float16)


def kernel(hidden_states, attention_mask, Wq, bq, Wk, bk, Wv, bv, dist_emb):
    nc = _build()
    from concourse import bass_utils

    hidden_states = np.asarray(hidden_states, np.float32)
    attention_mask = np.asarray(attention_mask, np.float32)
    Wq, Wk, Wv = (np.asarray(x, np.float32) for x in (Wq, Wk, Wv))
    bq, bk, bv = (np.asarray(x, np.float32) for x in (bq, bk, bv))
    dist_emb = np.asarray(dist_emb, np.float32)

    ETp = np.zeros((D, 2 * MAXPOS), np.float16)
    ETp[:, : 2 * MAXPOS - 1] = dist_emb.T.astype(np.float16)
    in_maps = []
    for c in range(NCORES):
        beta, g = c // 4, c % 4
        h = hidden_states[beta]
        rows = slice(256 * g, 256 * (g + 1))
        em = np.exp(attention_mask[beta, 0, 0, ::-1]).astype(np.float32)
        in_maps.append(
            {
                "hT": np.ascontiguousarray(h.T).astype(np.float16),
                "hTr": np.ascontiguousarray(h[::-1].T).astype(np.float16),
                "wqT": _swz(Wq[rows]),
                "wkT": _swz(Wk[rows]),
                "wvT": _swz(Wv[rows]),
                "bq": np.ascontiguousarray(bq[rows].reshape(MT, 128).T),
                "bk": np.ascontiguousarray(bk[rows].reshape(MT, 128).T),
                "bv": np.ascontiguousarray(bv[rows].reshape(MT, 128).T),
                "ET": ETp,
                "emr": np.ascontiguousarray(em.reshape(AT, 128).T),
            }
        )

    res = bass_utils.run_bass_kernel_spmd(nc, in_maps, list(range(NCORES)))
    out = np.empty((B, S, HID), np.float32)
    for c in range(NCORES):
        beta, g = c // 4, c % 4
        out[beta, :, 256 * g : 256 * (g + 1)] = res.results[c]["ctxT"].T
    return out



# revision 2
# speedup vs baseline: 3136.4197x; 3136.4197x over previous
"""BertSelfAttention (relative_key_query) Trainium2 Bass kernel, 8-core SPMD.

Sharding: 32 (batch, head) pairs -> core c handles batch c//4, heads
[4*(c%4), 4*(c%4)+4).  Each core runs an identical program on its own
input shard; host assembles the full [2, 2048, 1024] output.

Math (per core, per head), in "mirrored" coordinates a = 2047 - r, b = l:
  scores[a, b] = k'[a].q[b] + (k'[a] + q[b]) . E[a + b]           (E = dist_emb)
  probs        = exp(scores/8) * em'[a]  (em' = exp(mask) reversed), normalized
  ctx[b, d]    = sum_a probs[a, b] v'[a, d]                       (v' = v reversed)

The relative-position terms use the band-matmul + HBM "shear" trick:
  band[i, m] = src[A0+i] . E[A0+m]   (a plain matmul, [128 x 2175] per tile)
written to scratch DRAM with row stride 2176; the shifted-diagonal view
  rel[i, x] = band[i, i+x]  (per-partition shift)
is then a *plain strided read*: flat offset i*2177 + x.
"""

import sys

sys.path.insert(0, "/opt/trn_rl_repo")

import numpy as np

B, S, HID = 2, 2048, 1024
H, D = 16, 64
MAXPOS = 2048
NCORES = 8
HPC = 4  # heads per core

KT = 8  # 1024 / 128 contraction chunks for projections
MT = 2  # 256 / 128 output chunks for projections
NB = 4  # 2048 / 512 b-chunks
AT = 16  # 2048 / 128 a-tiles
BW = 2176  # band width (padded even; valid data in [0, 2175))
BR = 2176  # band row stride in scratch (shear: read stride BR+1)
BCHS = [512, 512, 512, 512, 128]  # band N-chunks (even sizes for fp32r)

_CACHE = {}


def _build():
    if "nc" in _CACHE:
        return _CACHE["nc"]

    import concourse.bass as bass
    import concourse.tile as tile
    from concourse import bacc, mybir
    from concourse.masks import make_identity

    dt = mybir.dt
    f32, bf16, fp8 = dt.float32, dt.bfloat16, dt.float8e4
    fp16 = dt.float16
    AF = mybir.ActivationFunctionType

    nc = bacc.Bacc(
        "TRN2", target_bir_lowering=False, debug=False, enable_asserts=True
    )

    hT = nc.declare_dram_parameter("hT", [HID, S], fp16, isOutput=False)
    hTr = nc.declare_dram_parameter("hTr", [HID, S], fp16, isOutput=False)
    # host pre-swizzled: [128, KT*256], chunk kc at cols [256*kc, 256*(kc+1))
    wqT = nc.declare_dram_parameter("wqT", [128, KT * 256], fp16, isOutput=False)
    wkT = nc.declare_dram_parameter("wkT", [128, KT * 256], fp16, isOutput=False)
    wvT = nc.declare_dram_parameter("wvT", [128, KT * 256], fp16, isOutput=False)
    bq = nc.declare_dram_parameter("bq", [128, MT], f32, isOutput=False)
    bk = nc.declare_dram_parameter("bk", [128, MT], f32, isOutput=False)
    bv = nc.declare_dram_parameter("bv", [128, MT], f32, isOutput=False)
    ET = nc.declare_dram_parameter("ET", [D, 2 * MAXPOS], fp16, isOutput=False)
    emr = nc.declare_dram_parameter("emr", [128, AT], f32, isOutput=False)
    ctxT = nc.declare_dram_parameter("ctxT", [256, S], f32, isOutput=True)

    # scratch: per head, per term: 16 tiles x [128 x BW] at row stride BR
    scr = [
        [nc.dram_tensor(f"scr_{h}_{t}", [AT * 128 * BR], fp8) for t in range(2)]
        for h in range(HPC)
    ]

    with tile.TileContext(nc) as tc:
        from contextlib import ExitStack

        with ExitStack() as ctx:
            persist = ctx.enter_context(tc.tile_pool(name="persist", bufs=1))

            # ---- constants ----
            # E^T duplicated into both partition halves so 64-row operands
            # based at partition 0 or 64 both find it at their own base
            et_sb = persist.tile([128, 2 * MAXPOS], fp16, tag="et")
            nc.sync.dma_start(et_sb[0:D, :], ET[:])
            nc.sync.dma_start(et_sb[D : 2 * D, :], ET[:])
            emr_sb = persist.tile([128, AT], f32, tag="emr")
            nc.sync.dma_start(emr_sb[:], emr[:])
            i128 = persist.tile([128, 128], fp16, tag="i128")
            make_identity(nc, i128[:])
            i128f = persist.tile([128, 128], fp16, tag="i128f")
            make_identity(nc, i128f[:])
            bias_sb = {}
            for nm, t in (("q", bq), ("k", bk), ("v", bv)):
                bias_sb[nm] = persist.tile([128, MT], f32, tag=f"b{nm}", name=f"bias_{nm}")
                nc.sync.dma_start(bias_sb[nm][:], t[:])

            # ---- projections: qT = wqT.T @ hT (+bias), etc. ----
            # [256, S] stored as [128, MT*S]: M-chunk m at cols [m*S, (m+1)*S)
            qT = persist.tile([128, MT * S], fp16, tag="qT")
            kT = persist.tile([128, MT * S], fp16, tag="kT")
            vT = persist.tile([128, MT * S], fp16, tag="vT")

            with tc.tile_pool(name="proj_ps", bufs=1, space="PSUM") as proj_ps, \
                 tc.tile_pool(name="hload", bufs=3) as hload:
                for nm, wt, src, dst in (
                    ("q", wqT, hT, qT),
                    ("k", wkT, hTr, kT),
                    ("v", wvT, hTr, vT),
                ):
                    # load full weight [1024, 256] as [128, 8*256] partition-major
                    w_sb = hload.tile([128, KT * 256], fp16, tag="w")
                    nc.sync.dma_start(w_sb[:], wt[:])
                    ps = [
                        [
                            proj_ps.tile(
                                [128, 512], mybir.dt.float32,
                                name=f"ps_{nm}_{m}_{n}", tag=f"ps_{m}_{n}",
                            )
                            for n in range(NB)
                        ]
                        for m in range(MT)
                    ]
                    for kc in range(KT):
                        h_sb = hload.tile([128, S], fp16, tag="h")
                        nc.sync.dma_start(h_sb[:], src[128 * kc : 128 * (kc + 1), :])
                        for m in range(MT):
                            for n in range(NB):
                                nc.tensor.matmul(
                                    ps[m][n][:],
                                    w_sb[:, kc * 256 + 128 * m : kc * 256 + 128 * (m + 1)],
                                    h_sb[:, 512 * n : 512 * (n + 1)],
                                    start=(kc == 0),
                                    stop=(kc == KT - 1),
                                )
                    for m in range(MT):
                        for n in range(NB):
                            # bias add (kept general; zero in practice)
                            nc.vector.tensor_scalar_add(
                                ps[m][n][:], ps[m][n][:], bias_sb[nm][:, m : m + 1]
                            )
                            d_ap = dst[:, m * S + 512 * n : m * S + 512 * (n + 1)]
                            if n % 2 == 0:
                                nc.vector.tensor_copy(d_ap, ps[m][n][:])
                            else:
                                nc.scalar.copy(d_ap, ps[m][n][:])

            # ---- per-head attention ----
            with tc.tile_pool(name="bpsum", bufs=2, space="PSUM") as bpsum, \
                 tc.tile_pool(name="spsum", bufs=2, space="PSUM") as spsum, \
                 tc.tile_pool(name="ctxps", bufs=1, space="PSUM") as ctxps, \
                 tc.tile_pool(name="bstage", bufs=3) as bstage, \
                 tc.tile_pool(name="termA", bufs=2) as termA_pool, \
                 tc.tile_pool(name="termB", bufs=1) as termB_pool, \
                 tc.tile_pool(name="probs", bufs=2) as probs_pool, \
                 tc.tile_pool(name="vaug", bufs=2) as vaug_pool, \
                 tc.tile_pool(name="epi", bufs=4) as epi:
                for hl in range(HPC):
                    p0 = 64 * (hl % 2)
                    c0 = (hl // 2) * S
                    qh = qT[p0 : p0 + 64, c0 : c0 + S]
                    kh = kT[p0 : p0 + 64, c0 : c0 + S]
                    vh = vT[p0 : p0 + 64, c0 : c0 + S]

                    # v_aug[a, 0:64] = v'[a, :] * em'[a];  v_aug[a, 64] = em'[a]
                    vaug = vaug_pool.tile([128, AT * 65], fp16, tag="vaug")
                    for at in range(AT):
                        vps = bpsum.tile([128, 512], fp16, tag="bp", name=f"vps_{hl}_{at}")
                        nc.tensor.transpose(
                            vps[:, 0:D],
                            vh[:, 128 * at : 128 * (at + 1)],
                            i128f[p0 : p0 + D, p0 : p0 + D],
                        )
                        nc.vector.tensor_scalar_mul(
                            vaug[:, 65 * at : 65 * at + 64],
                            vps[:, 0:D],
                            emr_sb[:, at : at + 1],
                        )
                        nc.vector.tensor_copy(
                            vaug[:, 65 * at + 64 : 65 * at + 65],
                            emr_sb[:, at : at + 1],
                        )

                    # bands -> scratch (term 0: k'-band, term 1: q-band)
                    for term, srcT in ((0, kh), (1, qh)):
                        for t in range(AT):
                            stage = bstage.tile([128, BW], fp16, tag="bs")
                            a0 = 128 * t
                            c0 = 0
                            for chi, bch in enumerate(BCHS):
                                bps = bpsum.tile([128, 512], mybir.dt.float32, tag="bp")
                                nc.tensor.matmul(
                                    bps[:, 0:bch],
                                    srcT[:, a0 : a0 + 128],
                                    et_sb[p0 : p0 + D, a0 + c0 : a0 + c0 + bch],
                                    start=True,
                                    stop=True,
                                )
                                if chi % 2 == 0:
                                    nc.vector.tensor_copy(
                                        stage[:, c0 : c0 + bch], bps[:, 0:bch]
                                    )
                                else:
                                    nc.scalar.copy(
                                        stage[:, c0 : c0 + bch], bps[:, 0:bch]
                                    )
                                c0 += bch
                            nc.gpsimd.dma_start(
                                bass.AP(scr[hl][term], 128 * BR * t, [[BR, 128], [1, BW]]),
                                stage[:],
                            )

                    # term-B tiles resident for the whole a-loop
                    termB = termB_pool.tile([128, AT * 2048], fp8, tag="termB")
                    for j in range(AT):
                        nc.sync.dma_start(
                            termB[:, 2048 * j : 2048 * (j + 1)],
                            bass.AP(scr[hl][1], 128 * BR * j, [[BR + 1, 128], [1, 2048]]),
                        )

                    ctx_ps = [
                        ctxps.tile(
                            [65, 512], mybir.dt.float32,
                            name=f"ctx_{hl}_{n}", tag=f"ctx_{n}",
                        )
                        for n in range(NB)
                    ]
                    for at in range(AT):
                        a0 = 128 * at
                        tA = termA_pool.tile([128, 2048], fp8, tag="tA")
                        nc.sync.dma_start(
                            tA[:],
                            bass.AP(scr[hl][0], 128 * BR * at, [[BR + 1, 128], [1, 2048]]),
                        )
                        probs = probs_pool.tile([128, 2048], fp16, tag="pr")
                        for n in range(NB):
                            sp = spsum.tile([128, 512], mybir.dt.float32, tag="s")
                            # transposed rel-q blocks (start=True zeroes each slice)
                            for jj in range(4):
                                j = 4 * n + jj
                                nc.tensor.matmul(
                                    sp[:, 128 * jj : 128 * (jj + 1)],
                                    termB[:, 2048 * j + a0 : 2048 * j + a0 + 128],
                                    i128[:],
                                    start=True,
                                    stop=False,
                                    skip_group_check=True,
                                )
                            # + rel-k (aligned)
                            nc.tensor.matmul(
                                sp[:],
                                i128[:],
                                tA[:, 512 * n : 512 * (n + 1)],
                                start=False,
                                stop=False,
                                skip_group_check=True,
                            )
                            # + k'.q
                            nc.tensor.matmul(
                                sp[:],
                                kh[:, a0 : a0 + 128],
                                qh[:, 512 * n : 512 * (n + 1)],
                                start=False,
                                stop=True,
                                skip_group_check=True,
                            )
                            nc.scalar.activation(
                                probs[:, 512 * n : 512 * (n + 1)],
                                sp[:],
                                AF.Exp,
                                scale=0.125,
                            )
                            nc.tensor.matmul(
                                ctx_ps[n][:],
                                vaug[:, 65 * at : 65 * (at + 1)],
                                probs[:, 512 * n : 512 * (n + 1)],
                                start=(at == 0),
                                stop=(at == AT - 1),
                            )

                    # normalize + write out
                    for n in range(NB):
                        den = epi.tile([1, 512], f32, tag="den")
                        nc.vector.tensor_copy(den[:], ctx_ps[n][64:65, :])
                        rec = epi.tile([1, 512], f32, tag="rec")
                        nc.vector.reciprocal(rec[:], den[:])
                        rec64 = epi.tile([64, 512], f32, tag="rec64")
                        nc.gpsimd.partition_broadcast(rec64[:], rec[:])
                        o = epi.tile([64, 512], f32, tag="o")
                        nc.vector.tensor_tensor(
                            o[:],
                            ctx_ps[n][0:64, :],
                            rec64[:],
                            mybir.AluOpType.mult,
                        )
                        nc.sync.dma_start(
                            ctxT[64 * hl : 64 * (hl + 1), 512 * n : 512 * (n + 1)],
                            o[:],
                        )

    nc.compile()
    _CACHE["nc"] = nc
    return nc


def _swz(w):
    # W block [256, 1024] -> W.T [1024, 256] -> [8, 128, 256] -> [128, 8*256]
    wT = np.ascontiguousarray(w.T).reshape(KT, 128, 256)
    return np.ascontiguousarray(wT.transpose(1, 0, 2).reshape(128, KT * 256)).astype(np.float16)


def kernel(hidden_states, attention_mask, Wq, bq, Wk, bk, Wv, bv, dist_emb):
    nc = _build()
    from concourse import bass_utils

    hidden_states = np.asarray(hidden_states, np.float32)
    attention_mask = np.asarray(attention_mask, np.float32)
    Wq, Wk, Wv = (np.asarray(x, np.float32) for x in (Wq, Wk, Wv))
    bq, bk, bv = (np.asarray(x, np.float32) for x in (bq, bk, bv))
    dist_emb = np.asarray(dist_emb, np.float32)

    ETp = np.zeros((D, 2 * MAXPOS), np.float16)
    ETp[:, : 2 * MAXPOS - 1] = dist_emb.T.astype(np.float16)
    in_maps = []
    for c in range(NCORES):
        beta, g = c // 4, c % 4
        h = hidden_states[beta]
        rows = slice(256 * g, 256 * (g + 1))
        em = np.exp(attention_mask[beta, 0, 0, ::-1]).astype(np.float32)
        in_maps.append(
            {
                "hT": np.ascontiguousarray(h.T).astype(np.float16),
                "hTr": np.ascontiguousarray(h[::-1].T).astype(np.float16),
                "wqT": _swz(Wq[rows]),
                "wkT": _swz(Wk[rows]),
                "wvT": _swz(Wv[rows]),
                "bq": np.ascontiguousarray(bq[rows].reshape(MT, 128).T),
                "bk": np.ascontiguousarray(bk[rows].reshape(MT, 128).T),
                "bv": np.ascontiguousarray(bv[rows].reshape(MT, 128).T),
                "ET": ETp,
                "emr": np.ascontiguousarray(em.reshape(AT, 128).T),
            }
        )

    res = bass_utils.run_bass_kernel_spmd(nc, in_maps, list(range(NCORES)))
    global LAST_RESULTS
    LAST_RESULTS = res
    out = np.empty((B, S, HID), np.float32)
    for c in range(NCORES):
        beta, g = c // 4, c % 4
        out[beta, :, 256 * g : 256 * (g + 1)] = res.results[c]["ctxT"].T
    return out

